# revision 10
# baseline (speedup 1.0000x reference)
"""DGM block (GCN conv -> pairwise sq-dist -> Gumbel top-k) on 8 TRN2 NeuronCores.

Self-contained: hardcodes the problem shapes (N=12288, D=256, K=4, 8 cores).

Algorithm (per core, SPMD; core c owns rows m in [c*M, (c+1)*M), M = N/8):
  phase 1 (GCN):  s = dinv * x (host),  split s = s_hi + s_lo (fp16 pair)
                  yT = s^T @ A01T_slice      (fp16 matmuls, fp32 PSUM; A01 exact in fp16)
                  vT = W^T @ yT              (fp32 matmul)
                  hT = vT * dinv_slice + b   -> h output (PE transpose)
  allgather:      fp16 split pair of hT plus -t*sq rows, one AllGather
  phase 2 (kNN):  z = 2t*h_m.h_n - t*sq_n   (3 fp16 split-matmul passes + aug row)
                  lq = min(z - t*sq_m, 0) - log(-log(q + 1e-8))
                  top-8 via DVE max8/max_index -> take top-4

Precision: fp16 hi/lo splits give ~22-bit effective mantissa on the big
matmuls; validated (numpy simulation) to reproduce the fp32 reference's
top-4 indices exactly.
"""
import os
import numpy as np

from concourse import bacc, dve_ops, mybir, tile
from concourse.bass_utils import run_bass_kernel_spmd
from concourse.dve_spec import Spec, Src0, Src1, C0, Zero, minn, lower, _has_src1
from concourse.dve_uop import DveOpSpec


def _register_minsub():
    """Fused DVE op: out = min(in0 + s0, 0) - in1  (one pass instead of two)."""
    name = "MINSUB_ANT"
    for op in dve_ops.OPS:
        if op.name == name:
            return op
    spec = Spec(
        body=minn(Src0 + C0, Zero) - Src1,
        reference=lambda in0, in1, s0, s1, imm2:
            (np.minimum(in0.astype(np.float32) + s0, 0.0) - in1)
            .astype(np.float32),
    )
    opcode = dve_ops._CUSTOM_DVE_ROW_BASE + len(dve_ops.OPS)
    shas = {}
    for ver in ("v3", "v4"):
        try:
            u = lower(spec, ver=ver)
            shas[ver] = DveOpSpec(name=name, opcode=opcode, uops=u,
                                  rd1_en=_has_src1(spec)).sha(ver)
        except Exception:
            pass
    op = dve_ops.DveOp(name, spec, subdim=False, uops_sha=shas)
    dve_ops.OPS.append(op)
    dve_ops.CUSTOM_DVE_SPECS[name] = spec
    dve_ops._SUB_OPCODE_FOR_NAME[name] = opcode
    return op


MINSUB = _register_minsub()

N = 12288
D = 256
NC = 8
M = N // NC          # 1536 rows per core
KNN = 4
QW = 1024            # q/Gumbel chunk width per ACT op
CH = 512             # matmul n-chunk width (one PSUM bank)

F32 = mybir.dt.float32
F16 = mybir.dt.float16
U32 = mybir.dt.uint32
AF = mybir.ActivationFunctionType
OP = mybir.AluOpType

_LAST_RUN = {}       # test harness introspection


def build_program(n, m, two_t, neg_t, n_cores):
    """SPMD bass program for one core. n = total nodes, m = rows per core."""
    kt_n = n // 128          # k tiles over all nodes
    mt_n = m // 128          # m tiles per core
    ch_n1 = m // CH          # phase-1 n-chunks
    ch_n2 = n // CH          # phase-2 n-chunks
    q_n = n // QW            # q chunks per m tile

    nc = bacc.Bacc()

    p1 = nc.dram_tensor("p1", [n, 512 + m], F16, kind="ExternalInput")
    wt = nc.dram_tensor("wt", [D, D], F32, kind="ExternalInput")
    bvec = nc.dram_tensor("bvec", [D, 1], F32, kind="ExternalInput")
    dinvb = nc.dram_tensor("dinvb", [128, m], F32, kind="ExternalInput")
    qs = nc.dram_tensor("qs", [m, n], F32, kind="ExternalInput")
    h_out = nc.dram_tensor("h_out", [m, D], F32, kind="ExternalOutput")
    tv_out = nc.dram_tensor("tv_out", [m, 8], F32, kind="ExternalOutput")
    ti_out = nc.dram_tensor("ti_out", [m, 8], U32, kind="ExternalOutput")

    rg = [list(range(n_cores))]

    with tile.TileContext(nc) as tc:
        with tc.tile_pool(name="persist", bufs=1) as pc, \
             tc.tile_pool(name="dram", bufs=1, space="DRAM") as pdram:
            # ---- persistent small constants ----
            eps = pc.tile([128, 1], F32)
            nc.gpsimd.memset(eps[:], 1e-8)
            ones2 = pc.tile([2, 128], F16)
            nc.gpsimd.memset(ones2[:], 1.0)
            tsqm = pc.tile([128, mt_n], F32)
            thi = pc.tile([128, 2, m], F16)     # fp16 split of 2t*hT (lhsT src)
            tlo = pc.tile([128, 2, m], F16)
            # dummy Ln pulls the ACT table load to kernel start
            dum = pc.tile([128, 1], F32)
            nc.scalar.activation(dum[:], eps[:], AF.Ln, bias=0.0, scale=1.0)

            agin = pdram.tile([2, D + 1, m], F16)
            agout = pdram.tile([n_cores, 2, D + 1, m], F16, addr_space="Shared")

            # ================= phase 1: GCN =================
            with nc.named_scope("ph1"), tc.tile_pool(name="ph1", bufs=1) as ph1:
                ones128 = ph1.tile([128, 128], F32)
                nc.gpsimd.memset(ones128[:], 1.0)
                ident = ph1.tile([128, 128], F32)
                nc.gpsimd.affine_select(ident[:], ones128[:], [[-1, 128]],
                                        OP.is_equal, 0.0, base=0,
                                        channel_multiplier=1)
                wt_sb = ph1.tile([128, 2, D], F32)
                nc.sync.dma_start(wt_sb[:],
                                  wt[:].rearrange("(kt p) d -> p kt d", p=128))
                b_sb = ph1.tile([128, 2], F32)
                nc.sync.dma_start(
                    b_sb[:], bvec[:].rearrange("(dh p) one -> p (dh one)", p=128))
                dinv_sb = ph1.tile([128, m], F32)
                nc.sync.dma_start(dinv_sb[:], dinvb[:])
                hT_sb = ph1.tile([128, 2, m], F32)
                h_sb = ph1.tile([128, mt_n, D], F32)

                with tc.tile_pool(name="p1ld", bufs=2) as p1ld, \
                     tc.tile_pool(name="psY", bufs=1, space="PSUM") as psY:
                    py = [[psY.tile([128, CH], F32, name=f"py_{dh}_{c}")
                           for c in range(ch_n1)] for dh in range(2)]
                    for t in range(kt_n):
                        pt = p1ld.tile([128, 512 + m], F16, name="pt", bufs=4)
                        nc.sync.dma_start(pt[:], p1[t * 128:(t + 1) * 128, :])
                        for s in range(2):
                            for dh in range(2):
                                for c in range(ch_n1):
                                    nc.tensor.matmul(
                                        py[dh][c][:],
                                        pt[:, s * 256 + dh * 128:
                                           s * 256 + (dh + 1) * 128],
                                        pt[:, 512 + c * CH: 512 + (c + 1) * CH],
                                        start=(t == 0 and s == 0),
                                        stop=(t == kt_n - 1 and s == 1),
                                    )
                    yT_sb = p1ld.tile([128, 2, m], F32, name="yT", bufs=1)
                    for dh in range(2):
                        for c in range(ch_n1):
                            nc.scalar.copy(
                                yT_sb[:, dh, c * CH:(c + 1) * CH], py[dh][c][:])

                # vT = W^T @ yT (fp32), then hT = vT * dinv + b
                with tc.tile_pool(name="psV", bufs=1, space="PSUM") as psV:
                    pv = [[psV.tile([128, CH], F32, name=f"pv_{dh}_{c}")
                           for c in range(ch_n1)] for dh in range(2)]
                    for dh in range(2):
                        for c in range(ch_n1):
                            for kt in range(2):
                                nc.tensor.matmul(
                                    pv[dh][c][:],
                                    wt_sb[:, kt, dh * 128:(dh + 1) * 128],
                                    yT_sb[:, kt, c * CH:(c + 1) * CH],
                                    start=(kt == 0), stop=(kt == 1),
                                )
                    for dh in range(2):
                        for c in range(ch_n1):
                            nc.vector.tensor_tensor(
                                hT_sb[:, dh, c * CH:(c + 1) * CH],
                                pv[dh][c][:],
                                dinv_sb[:, c * CH:(c + 1) * CH],
                                OP.mult,
                            )
                for dh in range(2):
                    nc.vector.tensor_scalar(
                        hT_sb[:, dh, :], hT_sb[:, dh, :],
                        b_sb[:, dh:dh + 1], None, OP.add)

                # h = hT^T -> h_out
                with tc.tile_pool(name="psT", bufs=4, space="PSUM") as psT:
                    for i in range(mt_n):
                        for dh in range(2):
                            ptr = psT.tile([128, 128], F32, name="ptr", bufs=4)
                            nc.tensor.transpose(
                                ptr[:], hT_sb[:, dh, i * 128:(i + 1) * 128],
                                ident[:])
                            nc.scalar.copy(
                                h_sb[:, i, dh * 128:(dh + 1) * 128], ptr[:])
                for i in range(mt_n):
                    nc.sync.dma_start(h_out[i * 128:(i + 1) * 128, :],
                                      h_sb[:, i, :])

                # -t*sq per own row (partition layout)
                sqscr = ph1.tile([128, D], F32)
                for i in range(mt_n):
                    nc.scalar.activation(sqscr[:], h_sb[:, i, :], AF.Square,
                                         bias=0.0, scale=1.0,
                                         accum_out=tsqm[:, i:i + 1])
                nc.vector.tensor_scalar(tsqm[:], tsqm[:], neg_t, None, OP.mult)

                # -t*sq over own columns (free layout), fp16 split
                hsq = ph1.tile([128, 2, m], F32)
                nc.vector.tensor_tensor(hsq[:], hT_sb[:], hT_sb[:], OP.mult)
                tsqr = ph1.tile([1, m], F32)
                with tc.tile_pool(name="psQ", bufs=2, space="PSUM") as psQ:
                    for c in range(ch_n1):
                        pq = psQ.tile([1, CH], F32, name="pq", bufs=2)
                        for dh in range(2):
                            nc.tensor.matmul(pq[:], ones128[:, 0:1],
                                             hsq[:, dh, c * CH:(c + 1) * CH],
                                             start=(dh == 0), stop=(dh == 1))
                        nc.scalar.copy(tsqr[:, c * CH:(c + 1) * CH], pq[:])
                nc.vector.tensor_scalar(tsqr[:], tsqr[:], neg_t, None, OP.mult)
                tsq_hi = ph1.tile([1, m], F16)
                tsq_hi32 = ph1.tile([1, m], F32)
                tsq_lo = ph1.tile([1, m], F16)
                nc.scalar.copy(tsq_hi[:], tsqr[:])
                nc.vector.tensor_copy(tsq_hi32[:], tsq_hi[:])
                nc.vector.tensor_tensor(tsq_lo[:], tsqr[:], tsq_hi32[:],
                                        OP.subtract)

                # fp16 split of hT (allgather payload = phase-2 rhs)
                hi16 = ph1.tile([128, 2, m], F16)
                lo16 = ph1.tile([128, 2, m], F16)
                hi32 = ph1.tile([128, 2, m], F32)
                nc.scalar.copy(hi16[:], hT_sb[:])
                nc.vector.tensor_copy(hi32[:], hi16[:])
                nc.vector.tensor_tensor(lo16[:], hT_sb[:], hi32[:], OP.subtract)

                # fp16 split of 2t*hT (phase-2 lhsT; robust for any t)
                th32 = ph1.tile([128, 2, m], F32)
                nc.vector.tensor_scalar(th32[:], hT_sb[:], two_t, None, OP.mult)
                nc.scalar.copy(thi[:], th32[:])
                nc.vector.tensor_copy(hi32[:], thi[:])
                nc.vector.tensor_tensor(tlo[:], th32[:], hi32[:], OP.subtract)

                # stage contribution, allgather
                nc.sync.dma_start(
                    agin[0, 0:D, :].rearrange("(dh p) m -> p dh m", p=128),
                    hi16[:])
                nc.sync.dma_start(
                    agin[1, 0:D, :].rearrange("(dh p) m -> p dh m", p=128),
                    lo16[:])
                nc.sync.dma_start(agin[0, D:D + 1, :], tsq_hi[:])
                nc.sync.dma_start(agin[1, D:D + 1, :], tsq_lo[:])
                with nc.named_scope("ag"):
                    nc.gpsimd.collective_compute(
                        "AllGather", OP.bypass, replica_groups=rg,
                        ins=[agin[:]], outs=[agout[:]],
                    )

            # ================= phase 2 =================
            with nc.named_scope("ph2"), \
                 tc.tile_pool(name="ph2", bufs=1) as ph2, \
                 tc.tile_pool(name="psZ", bufs=7, space="PSUM") as psZ, \
                 tc.tile_pool(name="psJ", bufs=1, space="PSUM") as psJ, \
                 tc.tile_pool(name="qp", bufs=2) as qp, \
                 tc.tile_pool(name="lp", bufs=2) as lp, \
                 tc.tile_pool(name="outp", bufs=2) as outp:
                rh = ph2.tile([128, 2, 2, n], F16)        # [p, split, dh, n]
                for s in range(2):
                    for dh in range(2):
                        nc.sync.dma_start(
                            rh[:, s, dh, :],
                            agout[:, s, dh * 128:(dh + 1) * 128, :]
                            .rearrange("c p m -> p c m"))
                augr = ph2.tile([2, n], F16)
                nc.sync.dma_start(augr[:],
                                  agout[:, :, D, :].rearrange("c s m -> s c m"))
                lq = ph2.tile([128, n], F32)
                junk = psJ.tile([128, CH], F32)
                # PE-warmth fillers: bridge the AllGather gap so the HAM
                # clock-gate stays at full rate into phase 2.
                for _ in range(96):
                    nc.tensor.matmul(junk[:], thi[:, 0, 0:128],
                                     thi[:, 0, 0:CH], start=True, stop=True)
                GC = 4                      # chunks per stationary-reuse group
                for i in range(mt_n):
                    for g in range(ch_n2 // GC):
                        tqs = []
                        for qq in range(GC * CH // QW):
                            qc = (g * GC) // (QW // CH) + qq
                            tq = qp.tile([128, QW], F32, name="tq", bufs=2)
                            nc.sync.dma_start(
                                tq[:],
                                qs[i * 128:(i + 1) * 128,
                                   qc * QW:(qc + 1) * QW])
                            tL = lp.tile([128, QW], F32, name="tL", bufs=2)
                            nc.scalar.activation(tL[:], tq[:], AF.Ln,
                                                 bias=eps[:], scale=1.0)
                            # g2 reuses the tq tile (tq dead after tL)
                            nc.scalar.activation(tq[:], tL[:], AF.Ln,
                                                 bias=0.0, scale=-1.0)
                            tqs.append(tq)
                        pzs = [psZ.tile([128, CH], F32, name="pz", bufs=7)
                               for _ in range(GC)]
                        # stationary-operand-outer order: one LDWEIGHTS per
                        # (lhs,dh) covering all GC chunks and both rh splits
                        # where applicable
                        for li, (lhs, srs) in enumerate(
                                ((thi, (0, 1)), (tlo, (0,)))):
                            for dh in range(2):
                                for sr in srs:
                                    for j in range(GC):
                                        c = g * GC + j
                                        nc.tensor.matmul(
                                            pzs[j][:],
                                            lhs[:, dh, i * 128:(i + 1) * 128],
                                            rh[:, sr, dh,
                                               c * CH:(c + 1) * CH],
                                            start=(li == 0 and dh == 0
                                                   and sr == 0),
                                            stop=False)
                        for j in range(GC):
                            c = g * GC + j
                            nc.tensor.matmul(pzs[j][:], ones2[:],
                                             augr[:, c * CH:(c + 1) * CH],
                                             start=False, stop=True)
                        for j in range(GC):
                            c = g * GC + j
                            half = c % (QW // CH)
                            tq = tqs[(j * CH) // QW]
                            # lq = min(z - t*sq_m, 0) - g2, one fused DVE pass
                            nc.vector._custom_dve(
                                MINSUB,
                                out=lq[:, c * CH:(c + 1) * CH],
                                in0=pzs[j][:],
                                in1=tq[:, half * CH:(half + 1) * CH],
                                s0=tsqm[:, i:i + 1])
                    tv8 = outp.tile([128, 8], F32, name="tv8", bufs=2)
                    ti8 = outp.tile([128, 8], U32, name="ti8", bufs=2)
                    nc.vector.max(tv8[:], lq[:])
                    nc.vector.max_index(ti8[:], tv8[:], lq[:])
                    nc.sync.dma_start(tv_out[i * 128:(i + 1) * 128, :], tv8[:])
                    nc.sync.dma_start(ti_out[i * 128:(i + 1) * 128, :], ti8[:])

    nc.finalize()
    return nc


def _host_prep(x, W, b, temperature, q, edge_index, n, m, n_cores):
    f32, f16 = np.float32, np.float16
    x = np.asarray(x, f32)
    W = np.asarray(W, f32)
    b = np.asarray(b, f32)
    q = np.asarray(q, f32)
    ei = np.asarray(edge_index).astype(np.int64)
    t = float(np.asarray(temperature))

    row = np.concatenate([ei[0], np.arange(n, dtype=np.int64)])  # dst
    col = np.concatenate([ei[1], np.arange(n, dtype=np.int64)])  # src
    deg = np.bincount(row, minlength=n).astype(f32)
    dinv = (1.0 / np.sqrt(np.maximum(deg, 1.0))).astype(f32)

    s = (dinv[:, None] * x).astype(f32)
    s_hi = s.astype(f16)
    s_lo = (s - s_hi.astype(f32)).astype(f16)

    at = np.zeros((n, n), dtype=np.uint8)        # AT[src, dst] multiplicity
    np.add.at(at, (col, row), 1)

    in_maps = []
    for c in range(n_cores):
        sl = slice(c * m, (c + 1) * m)
        p1c = np.empty((n, 512 + m), dtype=f16)
        p1c[:, 0:256] = s_hi
        p1c[:, 256:512] = s_lo
        p1c[:, 512:] = at[:, sl]
        in_maps.append({
            "p1": p1c,
            "wt": W,
            "bvec": b.reshape(D, 1),
            "dinvb": np.broadcast_to(dinv[sl][None, :], (128, m)).copy(),
            "qs": np.ascontiguousarray(q[sl, :]),
        })
    return in_maps, t


def kernel(x, W, b, temperature, q, edge_index):
    n, m, n_cores = N, M, NC
    in_maps, t = _host_prep(x, W, b, temperature, q, edge_index, n, m, n_cores)
    nc = build_program(n, m, two_t=2.0 * t, neg_t=-t, n_cores=n_cores)
    trace = bool(os.environ.get("DGM_TRACE"))
    res = run_bass_kernel_spmd(nc, in_maps, list(range(n_cores)), trace=trace)
    _LAST_RUN["res"] = res
    _LAST_RUN["nc"] = nc

    h = np.concatenate([res.results[c]["h_out"] for c in range(n_cores)], axis=0)
    tv8 = np.concatenate([res.results[c]["tv_out"] for c in range(n_cores)], axis=0)
    ti8 = np.concatenate([res.results[c]["ti_out"] for c in range(n_cores)], axis=0)
    top_vals = np.ascontiguousarray(tv8[:, :KNN]).astype(np.float32)
    top_idx = np.ascontiguousarray(ti8[:, :KNN]).astype(np.int32)
    edges = np.stack([top_idx.reshape(-1),
                      np.repeat(np.arange(n, dtype=np.int32), KNN)])
    return h, edges, top_vals


# revision 11
# speedup vs baseline: 1.0114x; 1.0114x over previous
"""DGM block (GCN conv -> pairwise sq-dist -> Gumbel top-k) on 8 TRN2 NeuronCores.

Self-contained: hardcodes the problem shapes (N=12288, D=256, K=4, 8 cores).

Algorithm (per core, SPMD; core c owns rows m in [c*M, (c+1)*M), M = N/8):
  phase 1 (GCN):  s = dinv * x (host),  split s = s_hi + s_lo (fp16 pair)
                  yT = s^T @ A01T_slice      (fp16 matmuls, fp32 PSUM; A01 exact in fp16)
                  vT = W^T @ yT              (fp32 matmul)
                  hT = vT * dinv_slice + b   -> h output (PE transpose)
  allgather:      fp16 split pair of hT plus -t*sq rows, one AllGather
  phase 2 (kNN):  z = 2t*h_m.h_n - t*sq_n   (3 fp16 split-matmul passes + aug row)
                  lq = min(z - t*sq_m, 0) - log(-log(q + 1e-8))
                  top-8 via DVE max8/max_index -> take top-4

Precision: fp16 hi/lo splits give ~22-bit effective mantissa on the big
matmuls; validated (numpy simulation) to reproduce the fp32 reference's
top-4 indices exactly.
"""
import os
import numpy as np

from concourse import bacc, dve_ops, mybir, tile
from concourse.bass_utils import run_bass_kernel_spmd
from concourse.dve_spec import Spec, Src0, Src1, C0, Zero, minn, lower, _has_src1
from concourse.dve_uop import DveOpSpec


def _register_minsub():
    """Fused DVE op: out = min(in0 + s0, 0) - in1  (one pass instead of two)."""
    name = "MINSUB_ANT"
    for op in dve_ops.OPS:
        if op.name == name:
            return op
    spec = Spec(
        body=minn(Src0 + C0, Zero) - Src1,
        reference=lambda in0, in1, s0, s1, imm2:
            (np.minimum(in0.astype(np.float32) + s0, 0.0) - in1)
            .astype(np.float32),
    )
    opcode = dve_ops._CUSTOM_DVE_ROW_BASE + len(dve_ops.OPS)
    shas = {}
    for ver in ("v3", "v4"):
        try:
            u = lower(spec, ver=ver)
            shas[ver] = DveOpSpec(name=name, opcode=opcode, uops=u,
                                  rd1_en=_has_src1(spec)).sha(ver)
        except Exception:
            pass
    op = dve_ops.DveOp(name, spec, subdim=False, uops_sha=shas)
    dve_ops.OPS.append(op)
    dve_ops.CUSTOM_DVE_SPECS[name] = spec
    dve_ops._SUB_OPCODE_FOR_NAME[name] = opcode
    return op


MINSUB = _register_minsub()

N = 12288
D = 256
NC = 8
M = N // NC          # 1536 rows per core
KNN = 4
QW = 1024            # q/Gumbel chunk width per ACT op
CH = 512             # matmul n-chunk width (one PSUM bank)

F32 = mybir.dt.float32
F16 = mybir.dt.float16
U32 = mybir.dt.uint32
AF = mybir.ActivationFunctionType
OP = mybir.AluOpType

_LAST_RUN = {}       # test harness introspection


def build_program(n, m, two_t, neg_t, n_cores):
    """SPMD bass program for one core. n = total nodes, m = rows per core."""
    kt_n = n // 128          # k tiles over all nodes
    mt_n = m // 128          # m tiles per core
    ch_n1 = m // CH          # phase-1 n-chunks
    ch_n2 = n // CH          # phase-2 n-chunks
    q_n = n // QW            # q chunks per m tile

    nc = bacc.Bacc()

    p1 = nc.dram_tensor("p1", [n, 512 + m], F16, kind="ExternalInput")
    wt = nc.dram_tensor("wt", [D, D], F32, kind="ExternalInput")
    bvec = nc.dram_tensor("bvec", [D, 1], F32, kind="ExternalInput")
    dinvb = nc.dram_tensor("dinvb", [128, m], F32, kind="ExternalInput")
    qs = nc.dram_tensor("qs", [m, n], F32, kind="ExternalInput")
    h_out = nc.dram_tensor("h_out", [m, D], F32, kind="ExternalOutput")
    tv_out = nc.dram_tensor("tv_out", [m, 8], F32, kind="ExternalOutput")
    ti_out = nc.dram_tensor("ti_out", [m, 8], U32, kind="ExternalOutput")

    rg = [list(range(n_cores))]

    with tile.TileContext(nc) as tc:
        with tc.tile_pool(name="persist", bufs=1) as pc, \
             tc.tile_pool(name="dram", bufs=1, space="DRAM") as pdram:
            # ---- persistent small constants ----
            eps = pc.tile([128, 1], F32)
            nc.gpsimd.memset(eps[:], 1e-8)
            ones2 = pc.tile([2, 128], F16)
            nc.gpsimd.memset(ones2[:], 1.0)
            tsqm = pc.tile([128, mt_n], F32)
            thi = pc.tile([128, 2, m], F16)     # fp16 split of 2t*hT (lhsT src)
            tlo = pc.tile([128, 2, m], F16)
            # dummy Ln pulls the ACT table load to kernel start
            dum = pc.tile([128, 1], F32)
            nc.scalar.activation(dum[:], eps[:], AF.Ln, bias=0.0, scale=1.0)

            agin = pdram.tile([2, D + 1, m], F16)
            agout = pdram.tile([n_cores, 2, D + 1, m], F16, addr_space="Shared")

            # ================= phase 1: GCN =================
            with nc.named_scope("ph1"), tc.tile_pool(name="ph1", bufs=1) as ph1:
                ones128 = ph1.tile([128, 128], F32)
                nc.gpsimd.memset(ones128[:], 1.0)
                ident = ph1.tile([128, 128], F32)
                nc.gpsimd.affine_select(ident[:], ones128[:], [[-1, 128]],
                                        OP.is_equal, 0.0, base=0,
                                        channel_multiplier=1)
                wt_sb = ph1.tile([128, 2, D], F32)
                nc.sync.dma_start(wt_sb[:],
                                  wt[:].rearrange("(kt p) d -> p kt d", p=128))
                b_sb = ph1.tile([128, 2], F32)
                nc.sync.dma_start(
                    b_sb[:], bvec[:].rearrange("(dh p) one -> p (dh one)", p=128))
                dinv_sb = ph1.tile([128, m], F32)
                nc.sync.dma_start(dinv_sb[:], dinvb[:])
                hT_sb = ph1.tile([128, 2, m], F32)
                h_sb = ph1.tile([128, mt_n, D], F32)

                with tc.tile_pool(name="p1ld", bufs=2) as p1ld, \
                     tc.tile_pool(name="psY", bufs=1, space="PSUM") as psY:
                    py = [[psY.tile([128, CH], F32, name=f"py_{dh}_{c}")
                           for c in range(ch_n1)] for dh in range(2)]
                    for t in range(kt_n):
                        pt = p1ld.tile([128, 512 + m], F16, name="pt", bufs=4)
                        nc.sync.dma_start(pt[:], p1[t * 128:(t + 1) * 128, :])
                        for s in range(2):
                            for dh in range(2):
                                for c in range(ch_n1):
                                    nc.tensor.matmul(
                                        py[dh][c][:],
                                        pt[:, s * 256 + dh * 128:
                                           s * 256 + (dh + 1) * 128],
                                        pt[:, 512 + c * CH: 512 + (c + 1) * CH],
                                        start=(t == 0 and s == 0),
                                        stop=(t == kt_n - 1 and s == 1),
                                    )
                    yT_sb = p1ld.tile([128, 2, m], F32, name="yT", bufs=1)
                    for dh in range(2):
                        for c in range(ch_n1):
                            nc.scalar.copy(
                                yT_sb[:, dh, c * CH:(c + 1) * CH], py[dh][c][:])

                # vT = W^T @ yT (fp32), then hT = vT * dinv + b
                with tc.tile_pool(name="psV", bufs=1, space="PSUM") as psV:
                    pv = [[psV.tile([128, CH], F32, name=f"pv_{dh}_{c}")
                           for c in range(ch_n1)] for dh in range(2)]
                    for dh in range(2):
                        for c in range(ch_n1):
                            for kt in range(2):
                                nc.tensor.matmul(
                                    pv[dh][c][:],
                                    wt_sb[:, kt, dh * 128:(dh + 1) * 128],
                                    yT_sb[:, kt, c * CH:(c + 1) * CH],
                                    start=(kt == 0), stop=(kt == 1),
                                )
                    for dh in range(2):
                        for c in range(ch_n1):
                            nc.vector.tensor_tensor(
                                hT_sb[:, dh, c * CH:(c + 1) * CH],
                                pv[dh][c][:],
                                dinv_sb[:, c * CH:(c + 1) * CH],
                                OP.mult,
                            )
                for dh in range(2):
                    nc.vector.tensor_scalar(
                        hT_sb[:, dh, :], hT_sb[:, dh, :],
                        b_sb[:, dh:dh + 1], None, OP.add)

                # h = hT^T -> h_out
                with tc.tile_pool(name="psT", bufs=4, space="PSUM") as psT:
                    for i in range(mt_n):
                        for dh in range(2):
                            ptr = psT.tile([128, 128], F32, name="ptr", bufs=4)
                            nc.tensor.transpose(
                                ptr[:], hT_sb[:, dh, i * 128:(i + 1) * 128],
                                ident[:])
                            nc.scalar.copy(
                                h_sb[:, i, dh * 128:(dh + 1) * 128], ptr[:])
                for i in range(mt_n):
                    nc.sync.dma_start(h_out[i * 128:(i + 1) * 128, :],
                                      h_sb[:, i, :])

                # -t*sq per own row (partition layout)
                sqscr = ph1.tile([128, D], F32)
                for i in range(mt_n):
                    nc.scalar.activation(sqscr[:], h_sb[:, i, :], AF.Square,
                                         bias=0.0, scale=1.0,
                                         accum_out=tsqm[:, i:i + 1])
                nc.vector.tensor_scalar(tsqm[:], tsqm[:], neg_t, None, OP.mult)

                # -t*sq over own columns (free layout), fp16 split
                hsq = ph1.tile([128, 2, m], F32)
                nc.vector.tensor_tensor(hsq[:], hT_sb[:], hT_sb[:], OP.mult)
                tsqr = ph1.tile([1, m], F32)
                with tc.tile_pool(name="psQ", bufs=2, space="PSUM") as psQ:
                    for c in range(ch_n1):
                        pq = psQ.tile([1, CH], F32, name="pq", bufs=2)
                        for dh in range(2):
                            nc.tensor.matmul(pq[:], ones128[:, 0:1],
                                             hsq[:, dh, c * CH:(c + 1) * CH],
                                             start=(dh == 0), stop=(dh == 1))
                        nc.scalar.copy(tsqr[:, c * CH:(c + 1) * CH], pq[:])
                nc.vector.tensor_scalar(tsqr[:], tsqr[:], neg_t, None, OP.mult)
                tsq_hi = ph1.tile([1, m], F16)
                tsq_hi32 = ph1.tile([1, m], F32)
                tsq_lo = ph1.tile([1, m], F16)
                nc.scalar.copy(tsq_hi[:], tsqr[:])
                nc.vector.tensor_copy(tsq_hi32[:], tsq_hi[:])
                nc.vector.tensor_tensor(tsq_lo[:], tsqr[:], tsq_hi32[:],
                                        OP.subtract)

                # fp16 split of hT (allgather payload = phase-2 rhs)
                hi16 = ph1.tile([128, 2, m], F16)
                lo16 = ph1.tile([128, 2, m], F16)
                hi32 = ph1.tile([128, 2, m], F32)
                nc.scalar.copy(hi16[:], hT_sb[:])
                nc.vector.tensor_copy(hi32[:], hi16[:])
                nc.vector.tensor_tensor(lo16[:], hT_sb[:], hi32[:], OP.subtract)

                # fp16 split of 2t*hT (phase-2 lhsT; robust for any t)
                th32 = ph1.tile([128, 2, m], F32)
                nc.vector.tensor_scalar(th32[:], hT_sb[:], two_t, None, OP.mult)
                nc.scalar.copy(thi[:], th32[:])
                nc.vector.tensor_copy(hi32[:], thi[:])
                nc.vector.tensor_tensor(tlo[:], th32[:], hi32[:], OP.subtract)

                # stage contribution, allgather
                nc.sync.dma_start(
                    agin[0, 0:D, :].rearrange("(dh p) m -> p dh m", p=128),
                    hi16[:])
                nc.sync.dma_start(
                    agin[1, 0:D, :].rearrange("(dh p) m -> p dh m", p=128),
                    lo16[:])
                nc.sync.dma_start(agin[0, D:D + 1, :], tsq_hi[:])
                nc.sync.dma_start(agin[1, D:D + 1, :], tsq_lo[:])
                with nc.named_scope("ag"):
                    nc.gpsimd.collective_compute(
                        "AllGather", OP.bypass, replica_groups=rg,
                        ins=[agin[:]], outs=[agout[:]],
                    )

            # ================= phase 2 =================
            with nc.named_scope("ph2"), \
                 tc.tile_pool(name="ph2", bufs=1) as ph2, \
                 tc.tile_pool(name="psZ", bufs=7, space="PSUM") as psZ, \
                 tc.tile_pool(name="psJ", bufs=1, space="PSUM") as psJ, \
                 tc.tile_pool(name="qp", bufs=2) as qp, \
                 tc.tile_pool(name="lp", bufs=2) as lp, \
                 tc.tile_pool(name="outp", bufs=2) as outp:
                rh = ph2.tile([128, 2, 2, n], F16)        # [p, split, dh, n]
                for s in range(2):
                    for dh in range(2):
                        nc.sync.dma_start(
                            rh[:, s, dh, :],
                            agout[:, s, dh * 128:(dh + 1) * 128, :]
                            .rearrange("c p m -> p c m"))
                augr = ph2.tile([2, n], F16)
                nc.sync.dma_start(augr[:],
                                  agout[:, :, D, :].rearrange("c s m -> s c m"))
                lq = ph2.tile([128, n], F32)
                junk = psJ.tile([128, CH], F32)
                # PE-warmth fillers: bridge the AllGather gap so the HAM
                # clock-gate stays at full rate into phase 2.
                for _ in range(96):
                    nc.tensor.matmul(junk[:], thi[:, 0, 0:128],
                                     thi[:, 0, 0:CH], start=True, stop=True)
                GC = 4                      # chunks per stationary-reuse group
                NQ = GC * CH // QW          # q tiles per group (2)
                for i in range(mt_n):
                    for g in range(ch_n2 // GC):
                        tqs = []
                        for qq in range(NQ):
                            qc = (g * GC) // (QW // CH) + qq
                            tq = qp.tile([128, QW], F32, name="tq", bufs=2)
                            nc.sync.dma_start(
                                tq[:],
                                qs[i * 128:(i + 1) * 128,
                                   qc * QW:(qc + 1) * QW])
                            tL = lp.tile([128, QW], F32, name="tL", bufs=2)
                            nc.scalar.activation(tL[:], tq[:], AF.Ln,
                                                 bias=eps[:], scale=1.0)
                            # g2 reuses the tq tile (tq dead after tL)
                            nc.scalar.activation(tq[:], tL[:], AF.Ln,
                                                 bias=0.0, scale=-1.0)
                            tqs.append(tq)
                        # double-bank psum tiles, one per q tile
                        pzs = [psZ.tile([128, QW], F32, name="pz", bufs=3)
                               for _ in range(NQ)]
                        # stationary-operand-outer order: one stationary load
                        # covers all GC chunks and both rh splits
                        for li, (lhs, srs) in enumerate(
                                ((thi, (0, 1)), (tlo, (0,)))):
                            for dh in range(2):
                                for sr in srs:
                                    for j in range(GC):
                                        c = g * GC + j
                                        pzv = pzs[(j * CH) // QW]
                                        off = (j * CH) % QW
                                        nc.tensor.matmul(
                                            pzv[:, off:off + CH],
                                            lhs[:, dh, i * 128:(i + 1) * 128],
                                            rh[:, sr, dh,
                                               c * CH:(c + 1) * CH],
                                            start=(li == 0 and dh == 0
                                                   and sr == 0),
                                            stop=False)
                        for j in range(GC):
                            c = g * GC + j
                            pzv = pzs[(j * CH) // QW]
                            off = (j * CH) % QW
                            nc.tensor.matmul(pzv[:, off:off + CH], ones2[:],
                                             augr[:, c * CH:(c + 1) * CH],
                                             start=False, stop=True)
                        for qq in range(NQ):
                            qc = (g * GC) // (QW // CH) + qq
                            # lq = min(z - t*sq_m, 0) - g2, one fused DVE pass
                            nc.vector._custom_dve(
                                MINSUB,
                                out=lq[:, qc * QW:(qc + 1) * QW],
                                in0=pzs[qq][:],
                                in1=tqs[qq][:],
                                s0=tsqm[:, i:i + 1])
                    tv8 = outp.tile([128, 8], F32, name="tv8", bufs=2)
                    ti8 = outp.tile([128, 8], U32, name="ti8", bufs=2)
                    nc.vector.max(tv8[:], lq[:])
                    nc.vector.max_index(ti8[:], tv8[:], lq[:])
                    nc.sync.dma_start(tv_out[i * 128:(i + 1) * 128, :], tv8[:])
                    nc.sync.dma_start(ti_out[i * 128:(i + 1) * 128, :], ti8[:])

    nc.finalize()
    return nc


def _host_prep(x, W, b, temperature, q, edge_index, n, m, n_cores):
    f32, f16 = np.float32, np.float16
    x = np.asarray(x, f32)
    W = np.asarray(W, f32)
    b = np.asarray(b, f32)
    q = np.asarray(q, f32)
    ei = np.asarray(edge_index).astype(np.int64)
    t = float(np.asarray(temperature))

    row = np.concatenate([ei[0], np.arange(n, dtype=np.int64)])  # dst
    col = np.concatenate([ei[1], np.arange(n, dtype=np.int64)])  # src
    deg = np.bincount(row, minlength=n).astype(f32)
    dinv = (1.0 / np.sqrt(np.maximum(deg, 1.0))).astype(f32)

    s = (dinv[:, None] * x).astype(f32)
    s_hi = s.astype(f16)
    s_lo = (s - s_hi.astype(f32)).astype(f16)

    at = np.zeros((n, n), dtype=np.uint8)        # AT[src, dst] multiplicity
    np.add.at(at, (col, row), 1)

    in_maps = []
    for c in range(n_cores):
        sl = slice(c * m, (c + 1) * m)
        p1c = np.empty((n, 512 + m), dtype=f16)
        p1c[:, 0:256] = s_hi
        p1c[:, 256:512] = s_lo
        p1c[:, 512:] = at[:, sl]
        in_maps.append({
            "p1": p1c,
            "wt": W,
            "bvec": b.reshape(D, 1),
            "dinvb": np.broadcast_to(dinv[sl][None, :], (128, m)).copy(),
            "qs": np.ascontiguousarray(q[sl, :]),
        })
    return in_maps, t


def kernel(x, W, b, temperature, q, edge_index):
    n, m, n_cores = N, M, NC
    in_maps, t = _host_prep(x, W, b, temperature, q, edge_index, n, m, n_cores)
    nc = build_program(n, m, two_t=2.0 * t, neg_t=-t, n_cores=n_cores)
    trace = bool(os.environ.get("DGM_TRACE"))
    res = run_bass_kernel_spmd(nc, in_maps, list(range(n_cores)), trace=trace)
    _LAST_RUN["res"] = res
    _LAST_RUN["nc"] = nc

    h = np.concatenate([res.results[c]["h_out"] for c in range(n_cores)], axis=0)
    tv8 = np.concatenate([res.results[c]["tv_out"] for c in range(n_cores)], axis=0)
    ti8 = np.concatenate([res.results[c]["ti_out"] for c in range(n_cores)], axis=0)
    top_vals = np.ascontiguousarray(tv8[:, :KNN]).astype(np.float32)
    top_idx = np.ascontiguousarray(ti8[:, :KNN]).astype(np.int32)
    edges = np.stack([top_idx.reshape(-1),
                      np.repeat(np.arange(n, dtype=np.int32), KNN)])
    return h, edges, top_vals


# revision 12
# speedup vs baseline: 1.0623x; 1.0502x over previous
"""DGM block (GCN conv -> pairwise sq-dist -> Gumbel top-k) on 8 TRN2 NeuronCores.

Self-contained: hardcodes the problem shapes (N=12288, D=256, K=4, 8 cores).

Algorithm (per core, SPMD; core c owns rows m in [c*M, (c+1)*M), M = N/8):
  phase 1 (GCN):  s = dinv * x (host),  split s = s_hi + s_lo (fp16 pair)
                  yT = s^T @ A01T_slice      (fp16 matmuls, fp32 PSUM; A01 exact in fp16)
                  vT = W^T @ yT              (fp32 matmul)
                  hT = vT * dinv_slice + b   -> h output (PE transpose)
  allgather:      fp16 split pair of hT plus -t*sq rows, one AllGather
  phase 2 (kNN):  z = 2t*h_m.h_n - t*sq_n   (3 fp16 split-matmul passes + aug row)
                  lq = min(z - t*sq_m, 0) - log(-log(q + 1e-8))
                  top-8 via DVE max8/max_index -> take top-4

Precision: fp16 hi/lo splits give ~22-bit effective mantissa on the big
matmuls; validated (numpy simulation) to reproduce the fp32 reference's
top-4 indices exactly.
"""
import os
import numpy as np

from concourse import bacc, dve_ops, mybir, tile
from concourse.bass_utils import run_bass_kernel_spmd
from concourse.dve_spec import Spec, Src0, Src1, C0, Zero, minn, lower, _has_src1
from concourse.dve_uop import DveOpSpec


def _register_minsub():
    """Fused DVE op: out = min(in0 + s0, 0) - in1  (one pass instead of two)."""
    name = "MINSUB_ANT"
    for op in dve_ops.OPS:
        if op.name == name:
            return op
    spec = Spec(
        body=minn(Src0 + C0, Zero) - Src1,
        reference=lambda in0, in1, s0, s1, imm2:
            (np.minimum(in0.astype(np.float32) + s0, 0.0) - in1)
            .astype(np.float32),
    )
    opcode = dve_ops._CUSTOM_DVE_ROW_BASE + len(dve_ops.OPS)
    shas = {}
    for ver in ("v3", "v4"):
        try:
            u = lower(spec, ver=ver)
            shas[ver] = DveOpSpec(name=name, opcode=opcode, uops=u,
                                  rd1_en=_has_src1(spec)).sha(ver)
        except Exception:
            pass
    op = dve_ops.DveOp(name, spec, subdim=False, uops_sha=shas)
    dve_ops.OPS.append(op)
    dve_ops.CUSTOM_DVE_SPECS[name] = spec
    dve_ops._SUB_OPCODE_FOR_NAME[name] = opcode
    return op


MINSUB = _register_minsub()

N = 12288
D = 256
NC = 8
M = N // NC          # 1536 rows per core
KNN = 4
QW = 1024            # q/Gumbel chunk width per ACT op
CH = 512             # matmul n-chunk width (one PSUM bank)

F32 = mybir.dt.float32
F16 = mybir.dt.float16
U32 = mybir.dt.uint32
AF = mybir.ActivationFunctionType
OP = mybir.AluOpType

_LAST_RUN = {}       # test harness introspection


def build_program(n, m, two_t, neg_t, n_cores):
    """SPMD bass program for one core. n = total nodes, m = rows per core."""
    kt_n = n // 128          # k tiles over all nodes
    mt_n = m // 128          # m tiles per core
    ch_n1 = m // CH          # phase-1 n-chunks
    ch_n2 = n // CH          # phase-2 n-chunks
    q_n = n // QW            # q chunks per m tile

    nc = bacc.Bacc()

    p1 = nc.dram_tensor("p1", [n, 512 + m], F16, kind="ExternalInput")
    wt = nc.dram_tensor("wt", [D, D], F32, kind="ExternalInput")
    bvec = nc.dram_tensor("bvec", [D, 1], F32, kind="ExternalInput")
    dinvb = nc.dram_tensor("dinvb", [128, m], F32, kind="ExternalInput")
    qs = nc.dram_tensor("qs", [m, n], F32, kind="ExternalInput")
    h_out = nc.dram_tensor("h_out", [m, D], F32, kind="ExternalOutput")
    tv_out = nc.dram_tensor("tv_out", [m, 8], F32, kind="ExternalOutput")
    ti_out = nc.dram_tensor("ti_out", [m, 8], U32, kind="ExternalOutput")

    rg = [list(range(n_cores))]

    with tile.TileContext(nc) as tc:
        with tc.tile_pool(name="persist", bufs=1) as pc, \
             tc.tile_pool(name="dram", bufs=1, space="DRAM") as pdram:
            # ---- persistent small constants ----
            eps = pc.tile([128, 1], F32)
            nc.gpsimd.memset(eps[:], 1e-8)
            ones2 = pc.tile([2, 128], F16)
            nc.gpsimd.memset(ones2[:], 1.0)
            tsqm = pc.tile([128, mt_n], F32)
            thi = pc.tile([128, 2, m], F16)     # fp16 split of 2t*hT (lhsT src)
            tlo = pc.tile([128, 2, m], F16)
            # dummy Ln pulls the ACT table load to kernel start
            dum = pc.tile([128, 1], F32)
            nc.scalar.activation(dum[:], eps[:], AF.Ln, bias=0.0, scale=1.0)

            agin = pdram.tile([2, D + 1, m], F16)
            agout = pdram.tile([n_cores, 2, D + 1, m], F16, addr_space="Shared")

            # ================= phase 1: GCN =================
            with nc.named_scope("ph1"), tc.tile_pool(name="ph1", bufs=1) as ph1:
                ones128 = ph1.tile([128, 128], F32)
                nc.gpsimd.memset(ones128[:], 1.0)
                ident = ph1.tile([128, 128], F32)
                nc.gpsimd.affine_select(ident[:], ones128[:], [[-1, 128]],
                                        OP.is_equal, 0.0, base=0,
                                        channel_multiplier=1)
                wt_sb = ph1.tile([128, 2, D], F32)
                nc.sync.dma_start(wt_sb[:],
                                  wt[:].rearrange("(kt p) d -> p kt d", p=128))
                b_sb = ph1.tile([128, 2], F32)
                nc.sync.dma_start(
                    b_sb[:], bvec[:].rearrange("(dh p) one -> p (dh one)", p=128))
                dinv_sb = ph1.tile([128, m], F32)
                nc.sync.dma_start(dinv_sb[:], dinvb[:])
                hT_sb = ph1.tile([128, 2, m], F32)
                h_sb = ph1.tile([128, mt_n, D], F32)

                with tc.tile_pool(name="p1ld", bufs=2) as p1ld, \
                     tc.tile_pool(name="psY", bufs=1, space="PSUM") as psY:
                    py = [[psY.tile([128, CH], F32, name=f"py_{dh}_{c}")
                           for c in range(ch_n1)] for dh in range(2)]
                    for t in range(kt_n):
                        pt = p1ld.tile([128, 512 + m], F16, name="pt", bufs=4)
                        nc.sync.dma_start(pt[:], p1[t * 128:(t + 1) * 128, :])
                        for s in range(2):
                            for dh in range(2):
                                for c in range(ch_n1):
                                    nc.tensor.matmul(
                                        py[dh][c][:],
                                        pt[:, s * 256 + dh * 128:
                                           s * 256 + (dh + 1) * 128],
                                        pt[:, 512 + c * CH: 512 + (c + 1) * CH],
                                        start=(t == 0 and s == 0),
                                        stop=(t == kt_n - 1 and s == 1),
                                    )
                    yT_sb = p1ld.tile([128, 2, m], F32, name="yT", bufs=1)
                    for dh in range(2):
                        for c in range(ch_n1):
                            nc.scalar.copy(
                                yT_sb[:, dh, c * CH:(c + 1) * CH], py[dh][c][:])

                # vT = W^T @ yT (fp32), then hT = vT * dinv + b
                with tc.tile_pool(name="psV", bufs=1, space="PSUM") as psV:
                    pv = [[psV.tile([128, CH], F32, name=f"pv_{dh}_{c}")
                           for c in range(ch_n1)] for dh in range(2)]
                    for dh in range(2):
                        for c in range(ch_n1):
                            for kt in range(2):
                                nc.tensor.matmul(
                                    pv[dh][c][:],
                                    wt_sb[:, kt, dh * 128:(dh + 1) * 128],
                                    yT_sb[:, kt, c * CH:(c + 1) * CH],
                                    start=(kt == 0), stop=(kt == 1),
                                )
                    for dh in range(2):
                        for c in range(ch_n1):
                            nc.vector.tensor_tensor(
                                hT_sb[:, dh, c * CH:(c + 1) * CH],
                                pv[dh][c][:],
                                dinv_sb[:, c * CH:(c + 1) * CH],
                                OP.mult,
                            )
                for dh in range(2):
                    nc.vector.tensor_scalar(
                        hT_sb[:, dh, :], hT_sb[:, dh, :],
                        b_sb[:, dh:dh + 1], None, OP.add)

                # h = hT^T -> h_out
                with tc.tile_pool(name="psT", bufs=4, space="PSUM") as psT:
                    for i in range(mt_n):
                        for dh in range(2):
                            ptr = psT.tile([128, 128], F32, name="ptr", bufs=4)
                            nc.tensor.transpose(
                                ptr[:], hT_sb[:, dh, i * 128:(i + 1) * 128],
                                ident[:])
                            nc.scalar.copy(
                                h_sb[:, i, dh * 128:(dh + 1) * 128], ptr[:])
                for i in range(mt_n):
                    nc.sync.dma_start(h_out[i * 128:(i + 1) * 128, :],
                                      h_sb[:, i, :])

                # -t*sq per own row (partition layout)
                sqscr = ph1.tile([128, D], F32)
                for i in range(mt_n):
                    nc.scalar.activation(sqscr[:], h_sb[:, i, :], AF.Square,
                                         bias=0.0, scale=1.0,
                                         accum_out=tsqm[:, i:i + 1])
                nc.vector.tensor_scalar(tsqm[:], tsqm[:], neg_t, None, OP.mult)

                # -t*sq over own columns (free layout), fp16 split
                hsq = ph1.tile([128, 2, m], F32)
                nc.vector.tensor_tensor(hsq[:], hT_sb[:], hT_sb[:], OP.mult)
                tsqr = ph1.tile([1, m], F32)
                with tc.tile_pool(name="psQ", bufs=2, space="PSUM") as psQ:
                    for c in range(ch_n1):
                        pq = psQ.tile([1, CH], F32, name="pq", bufs=2)
                        for dh in range(2):
                            nc.tensor.matmul(pq[:], ones128[:, 0:1],
                                             hsq[:, dh, c * CH:(c + 1) * CH],
                                             start=(dh == 0), stop=(dh == 1))
                        nc.scalar.copy(tsqr[:, c * CH:(c + 1) * CH], pq[:])
                nc.vector.tensor_scalar(tsqr[:], tsqr[:], neg_t, None, OP.mult)
                tsq_hi = ph1.tile([1, m], F16)
                tsq_hi32 = ph1.tile([1, m], F32)
                tsq_lo = ph1.tile([1, m], F16)
                nc.scalar.copy(tsq_hi[:], tsqr[:])
                nc.vector.tensor_copy(tsq_hi32[:], tsq_hi[:])
                nc.vector.tensor_tensor(tsq_lo[:], tsqr[:], tsq_hi32[:],
                                        OP.subtract)

                # fp16 split of hT (allgather payload = phase-2 rhs)
                hi16 = ph1.tile([128, 2, m], F16)
                lo16 = ph1.tile([128, 2, m], F16)
                hi32 = ph1.tile([128, 2, m], F32)
                nc.scalar.copy(hi16[:], hT_sb[:])
                nc.vector.tensor_copy(hi32[:], hi16[:])
                nc.vector.tensor_tensor(lo16[:], hT_sb[:], hi32[:], OP.subtract)

                # fp16 split of 2t*hT (phase-2 lhsT; robust for any t)
                th32 = ph1.tile([128, 2, m], F32)
                nc.vector.tensor_scalar(th32[:], hT_sb[:], two_t, None, OP.mult)
                nc.scalar.copy(thi[:], th32[:])
                nc.vector.tensor_copy(hi32[:], thi[:])
                nc.vector.tensor_tensor(tlo[:], th32[:], hi32[:], OP.subtract)

                # stage contribution, allgather
                nc.sync.dma_start(
                    agin[0, 0:D, :].rearrange("(dh p) m -> p dh m", p=128),
                    hi16[:])
                nc.sync.dma_start(
                    agin[1, 0:D, :].rearrange("(dh p) m -> p dh m", p=128),
                    lo16[:])
                nc.sync.dma_start(agin[0, D:D + 1, :], tsq_hi[:])
                nc.sync.dma_start(agin[1, D:D + 1, :], tsq_lo[:])
                with nc.named_scope("ag"):
                    nc.gpsimd.collective_compute(
                        "AllGather", OP.bypass, replica_groups=rg,
                        ins=[agin[:]], outs=[agout[:]],
                    )

            # ================= phase 2 =================
            with nc.named_scope("ph2"), \
                 tc.tile_pool(name="ph2", bufs=1) as ph2, \
                 tc.tile_pool(name="psZ", bufs=7, space="PSUM") as psZ, \
                 tc.tile_pool(name="psJ", bufs=1, space="PSUM") as psJ, \
                 tc.tile_pool(name="qp", bufs=2) as qp, \
                 tc.tile_pool(name="lp", bufs=2) as lp, \
                 tc.tile_pool(name="outp", bufs=2) as outp:
                rh = ph2.tile([128, 2, 2, n], F16)        # [p, split, dh, n]
                for s in range(2):
                    for dh in range(2):
                        nc.sync.dma_start(
                            rh[:, s, dh, :],
                            agout[:, s, dh * 128:(dh + 1) * 128, :]
                            .rearrange("c p m -> p c m"))
                augr = ph2.tile([2, n], F16)
                nc.sync.dma_start(augr[:],
                                  agout[:, :, D, :].rearrange("c s m -> s c m"))
                lq = ph2.tile([128, n], F32)
                junk = psJ.tile([128, CH], F32)
                # PE-warmth fillers: bridge the AllGather gap so the HAM
                # clock-gate stays at full rate into phase 2.
                for _ in range(144):
                    nc.tensor.matmul(junk[:], thi[:, 0, 0:128],
                                     thi[:, 0, 0:CH], start=True, stop=True)
                GC = 4                      # chunks per stationary-reuse group
                NQ = GC * CH // QW          # q tiles per group (2)
                for i in range(mt_n):
                    for g in range(ch_n2 // GC):
                        tqs = []
                        for qq in range(NQ):
                            qc = (g * GC) // (QW // CH) + qq
                            tq = qp.tile([128, QW], F32, name="tq", bufs=4)
                            nc.sync.dma_start(
                                tq[:],
                                qs[i * 128:(i + 1) * 128,
                                   qc * QW:(qc + 1) * QW])
                            tL = lp.tile([128, QW], F32, name="tL", bufs=2)
                            nc.scalar.activation(tL[:], tq[:], AF.Ln,
                                                 bias=eps[:], scale=1.0)
                            # g2 reuses the tq tile (tq dead after tL)
                            nc.scalar.activation(tq[:], tL[:], AF.Ln,
                                                 bias=0.0, scale=-1.0)
                            tqs.append(tq)
                        # double-bank psum tiles, one per q tile
                        pzs = [psZ.tile([128, QW], F32, name="pz", bufs=3)
                               for _ in range(NQ)]
                        # stationary-operand-outer order: one stationary load
                        # covers all GC chunks and both rh splits
                        for li, (lhs, srs) in enumerate(
                                ((thi, (0, 1)), (tlo, (0,)))):
                            for dh in range(2):
                                for sr in srs:
                                    for j in range(GC):
                                        c = g * GC + j
                                        pzv = pzs[(j * CH) // QW]
                                        off = (j * CH) % QW
                                        nc.tensor.matmul(
                                            pzv[:, off:off + CH],
                                            lhs[:, dh, i * 128:(i + 1) * 128],
                                            rh[:, sr, dh,
                                               c * CH:(c + 1) * CH],
                                            start=(li == 0 and dh == 0
                                                   and sr == 0),
                                            stop=False)
                        for j in range(GC):
                            c = g * GC + j
                            pzv = pzs[(j * CH) // QW]
                            off = (j * CH) % QW
                            nc.tensor.matmul(pzv[:, off:off + CH], ones2[:],
                                             augr[:, c * CH:(c + 1) * CH],
                                             start=False, stop=True)
                        for qq in range(NQ):
                            qc = (g * GC) // (QW // CH) + qq
                            # lq = min(z - t*sq_m, 0) - g2, one fused DVE pass
                            nc.vector._custom_dve(
                                MINSUB,
                                out=lq[:, qc * QW:(qc + 1) * QW],
                                in0=pzs[qq][:],
                                in1=tqs[qq][:],
                                s0=tsqm[:, i:i + 1])
                    tv8 = outp.tile([128, 8], F32, name="tv8", bufs=2)
                    ti8 = outp.tile([128, 8], U32, name="ti8", bufs=2)
                    nc.vector.max(tv8[:], lq[:])
                    nc.vector.max_index(ti8[:], tv8[:], lq[:])
                    nc.sync.dma_start(tv_out[i * 128:(i + 1) * 128, :], tv8[:])
                    nc.sync.dma_start(ti_out[i * 128:(i + 1) * 128, :], ti8[:])

    nc.finalize()
    return nc


def _host_prep(x, W, b, temperature, q, edge_index, n, m, n_cores):
    f32, f16 = np.float32, np.float16
    x = np.asarray(x, f32)
    W = np.asarray(W, f32)
    b = np.asarray(b, f32)
    q = np.asarray(q, f32)
    ei = np.asarray(edge_index).astype(np.int64)
    t = float(np.asarray(temperature))

    row = np.concatenate([ei[0], np.arange(n, dtype=np.int64)])  # dst
    col = np.concatenate([ei[1], np.arange(n, dtype=np.int64)])  # src
    deg = np.bincount(row, minlength=n).astype(f32)
    dinv = (1.0 / np.sqrt(np.maximum(deg, 1.0))).astype(f32)

    s = (dinv[:, None] * x).astype(f32)
    s_hi = s.astype(f16)
    s_lo = (s - s_hi.astype(f32)).astype(f16)

    at = np.zeros((n, n), dtype=np.uint8)        # AT[src, dst] multiplicity
    np.add.at(at, (col, row), 1)

    in_maps = []
    for c in range(n_cores):
        sl = slice(c * m, (c + 1) * m)
        p1c = np.empty((n, 512 + m), dtype=f16)
        p1c[:, 0:256] = s_hi
        p1c[:, 256:512] = s_lo
        p1c[:, 512:] = at[:, sl]
        in_maps.append({
            "p1": p1c,
            "wt": W,
            "bvec": b.reshape(D, 1),
            "dinvb": np.broadcast_to(dinv[sl][None, :], (128, m)).copy(),
            "qs": np.ascontiguousarray(q[sl, :]),
        })
    return in_maps, t


def kernel(x, W, b, temperature, q, edge_index):
    n, m, n_cores = N, M, NC
    in_maps, t = _host_prep(x, W, b, temperature, q, edge_index, n, m, n_cores)
    nc = build_program(n, m, two_t=2.0 * t, neg_t=-t, n_cores=n_cores)
    trace = bool(os.environ.get("DGM_TRACE"))
    res = run_bass_kernel_spmd(nc, in_maps, list(range(n_cores)), trace=trace)
    _LAST_RUN["res"] = res
    _LAST_RUN["nc"] = nc

    h = np.concatenate([res.results[c]["h_out"] for c in range(n_cores)], axis=0)
    tv8 = np.concatenate([res.results[c]["tv_out"] for c in range(n_cores)], axis=0)
    ti8 = np.concatenate([res.results[c]["ti_out"] for c in range(n_cores)], axis=0)
    top_vals = np.ascontiguousarray(tv8[:, :KNN]).astype(np.float32)
    top_idx = np.ascontiguousarray(ti8[:, :KNN]).astype(np.int32)
    edges = np.stack([top_idx.reshape(-1),
                      np.repeat(np.arange(n, dtype=np.int32), KNN)])
    return h, edges, top_vals


# revision 13
# speedup vs baseline: 1.0624x; 1.0001x over previous
"""DGM block (GCN conv -> pairwise sq-dist -> Gumbel top-k) on 8 TRN2 NeuronCores.

Self-contained: hardcodes the problem shapes (N=12288, D=256, K=4, 8 cores).

Algorithm (per core, SPMD; core c owns rows m in [c*M, (c+1)*M), M = N/8):
  phase 1 (GCN):  s = dinv * x (host),  split s = s_hi + s_lo (fp16 pair)
                  yT = s^T @ A01T_slice      (fp16 matmuls, fp32 PSUM; A01 exact in fp16)
                  vT = W^T @ yT              (fp32 matmul)
                  hT = vT * dinv_slice + b   -> h output (PE transpose)
  allgather:      fp16 split pair of hT plus -t*sq rows, one AllGather
  phase 2 (kNN):  z = 2t*h_m.h_n - t*sq_n   (3 fp16 split-matmul passes + aug row)
                  lq = min(z - t*sq_m, 0) - log(-log(q + 1e-8))
                  top-8 via DVE max8/max_index -> take top-4

Precision: fp16 hi/lo splits give ~22-bit effective mantissa on the big
matmuls; validated (numpy simulation) to reproduce the fp32 reference's
top-4 indices exactly.
"""
import os
import numpy as np

from concourse import bacc, dve_ops, mybir, tile
from concourse.bass_utils import run_bass_kernel_spmd
from concourse.dve_spec import Spec, Src0, Src1, C0, Zero, minn, lower, _has_src1
from concourse.dve_uop import DveOpSpec


def _register_minsub():
    """Fused DVE op: out = min(in0 + s0, 0) - in1  (one pass instead of two)."""
    name = "MINSUB_ANT"
    for op in dve_ops.OPS:
        if op.name == name:
            return op
    spec = Spec(
        body=minn(Src0 + C0, Zero) - Src1,
        reference=lambda in0, in1, s0, s1, imm2:
            (np.minimum(in0.astype(np.float32) + s0, 0.0) - in1)
            .astype(np.float32),
    )
    opcode = dve_ops._CUSTOM_DVE_ROW_BASE + len(dve_ops.OPS)
    shas = {}
    for ver in ("v3", "v4"):
        try:
            u = lower(spec, ver=ver)
            shas[ver] = DveOpSpec(name=name, opcode=opcode, uops=u,
                                  rd1_en=_has_src1(spec)).sha(ver)
        except Exception:
            pass
    op = dve_ops.DveOp(name, spec, subdim=False, uops_sha=shas)
    dve_ops.OPS.append(op)
    dve_ops.CUSTOM_DVE_SPECS[name] = spec
    dve_ops._SUB_OPCODE_FOR_NAME[name] = opcode
    return op


MINSUB = _register_minsub()

N = 12288
D = 256
NC = 8
M = N // NC          # 1536 rows per core
KNN = 4
QW = 1024            # q/Gumbel chunk width per ACT op
CH = 512             # matmul n-chunk width (one PSUM bank)

F32 = mybir.dt.float32
F16 = mybir.dt.float16
U32 = mybir.dt.uint32
AF = mybir.ActivationFunctionType
OP = mybir.AluOpType

_LAST_RUN = {}       # test harness introspection


def build_program(n, m, two_t, neg_t, n_cores):
    """SPMD bass program for one core. n = total nodes, m = rows per core."""
    kt_n = n // 128          # k tiles over all nodes
    mt_n = m // 128          # m tiles per core
    ch_n1 = m // CH          # phase-1 n-chunks
    ch_n2 = n // CH          # phase-2 n-chunks
    q_n = n // QW            # q chunks per m tile

    nc = bacc.Bacc()

    p1 = nc.dram_tensor("p1", [n, 512 + m], F16, kind="ExternalInput")
    wt = nc.dram_tensor("wt", [D, D], F32, kind="ExternalInput")
    bvec = nc.dram_tensor("bvec", [D, 1], F32, kind="ExternalInput")
    dinvb = nc.dram_tensor("dinvb", [128, m], F32, kind="ExternalInput")
    qs = nc.dram_tensor("qs", [m, n], F32, kind="ExternalInput")
    h_out = nc.dram_tensor("h_out", [m, D], F32, kind="ExternalOutput")
    tv_out = nc.dram_tensor("tv_out", [m, 8], F32, kind="ExternalOutput")
    ti_out = nc.dram_tensor("ti_out", [m, 8], U32, kind="ExternalOutput")

    rg = [list(range(n_cores))]

    with tile.TileContext(nc) as tc:
        with tc.tile_pool(name="persist", bufs=1) as pc, \
             tc.tile_pool(name="dram", bufs=1, space="DRAM") as pdram:
            # ---- persistent small constants ----
            eps = pc.tile([128, 1], F32)
            nc.gpsimd.memset(eps[:], 1e-8)
            ones2 = pc.tile([2, 128], F16)
            nc.gpsimd.memset(ones2[:], 1.0)
            tsqm = pc.tile([128, mt_n], F32)
            thi = pc.tile([128, 2, m], F16)     # fp16 split of 2t*hT (lhsT src)
            tlo = pc.tile([128, 2, m], F16)
            # dummy Ln pulls the ACT table load to kernel start
            dum = pc.tile([128, 1], F32)
            nc.scalar.activation(dum[:], eps[:], AF.Ln, bias=0.0, scale=1.0)

            agin = pdram.tile([2, D + 1, m], F16)
            agout = pdram.tile([n_cores, 2, D + 1, m], F16, addr_space="Shared")

            # ================= phase 1: GCN =================
            with nc.named_scope("ph1"), tc.tile_pool(name="ph1", bufs=1) as ph1:
                ones128 = ph1.tile([128, 128], F32)
                nc.gpsimd.memset(ones128[:], 1.0)
                ident = ph1.tile([128, 128], F32)
                nc.gpsimd.affine_select(ident[:], ones128[:], [[-1, 128]],
                                        OP.is_equal, 0.0, base=0,
                                        channel_multiplier=1)
                wt_sb = ph1.tile([128, 2, D], F32)
                nc.sync.dma_start(wt_sb[:],
                                  wt[:].rearrange("(kt p) d -> p kt d", p=128))
                b_sb = ph1.tile([128, 2], F32)
                nc.sync.dma_start(
                    b_sb[:], bvec[:].rearrange("(dh p) one -> p (dh one)", p=128))
                dinv_sb = ph1.tile([128, m], F32)
                nc.sync.dma_start(dinv_sb[:], dinvb[:])
                hT_sb = ph1.tile([128, 2, m], F32)
                h_sb = ph1.tile([128, mt_n, D], F32)

                with tc.tile_pool(name="p1ld", bufs=2) as p1ld, \
                     tc.tile_pool(name="psY", bufs=1, space="PSUM") as psY:
                    py = [[psY.tile([128, CH], F32, name=f"py_{dh}_{c}")
                           for c in range(ch_n1)] for dh in range(2)]
                    for t in range(kt_n):
                        pt = p1ld.tile([128, 512 + m], F16, name="pt", bufs=4)
                        nc.sync.dma_start(pt[:], p1[t * 128:(t + 1) * 128, :])
                        for s in range(2):
                            for dh in range(2):
                                for c in range(ch_n1):
                                    nc.tensor.matmul(
                                        py[dh][c][:],
                                        pt[:, s * 256 + dh * 128:
                                           s * 256 + (dh + 1) * 128],
                                        pt[:, 512 + c * CH: 512 + (c + 1) * CH],
                                        start=(t == 0 and s == 0),
                                        stop=(t == kt_n - 1 and s == 1),
                                    )
                    yT_sb = p1ld.tile([128, 2, m], F32, name="yT", bufs=1)
                    for dh in range(2):
                        for c in range(ch_n1):
                            nc.scalar.copy(
                                yT_sb[:, dh, c * CH:(c + 1) * CH], py[dh][c][:])

                # vT = W^T @ yT (fp32), then hT = vT * dinv + b
                with tc.tile_pool(name="psV", bufs=1, space="PSUM") as psV:
                    pv = [[psV.tile([128, CH], F32, name=f"pv_{dh}_{c}")
                           for c in range(ch_n1)] for dh in range(2)]
                    for dh in range(2):
                        for c in range(ch_n1):
                            for kt in range(2):
                                nc.tensor.matmul(
                                    pv[dh][c][:],
                                    wt_sb[:, kt, dh * 128:(dh + 1) * 128],
                                    yT_sb[:, kt, c * CH:(c + 1) * CH],
                                    start=(kt == 0), stop=(kt == 1),
                                )
                    for dh in range(2):
                        for c in range(ch_n1):
                            nc.vector.tensor_tensor(
                                hT_sb[:, dh, c * CH:(c + 1) * CH],
                                pv[dh][c][:],
                                dinv_sb[:, c * CH:(c + 1) * CH],
                                OP.mult,
                            )
                for dh in range(2):
                    nc.vector.tensor_scalar(
                        hT_sb[:, dh, :], hT_sb[:, dh, :],
                        b_sb[:, dh:dh + 1], None, OP.add)

                # h = hT^T -> h_out
                with tc.tile_pool(name="psT", bufs=4, space="PSUM") as psT:
                    for i in range(mt_n):
                        for dh in range(2):
                            ptr = psT.tile([128, 128], F32, name="ptr", bufs=4)
                            nc.tensor.transpose(
                                ptr[:], hT_sb[:, dh, i * 128:(i + 1) * 128],
                                ident[:])
                            nc.scalar.copy(
                                h_sb[:, i, dh * 128:(dh + 1) * 128], ptr[:])
                for i in range(mt_n):
                    nc.sync.dma_start(h_out[i * 128:(i + 1) * 128, :],
                                      h_sb[:, i, :])

                # -t*sq per own row (partition layout)
                sqscr = ph1.tile([128, D], F32)
                for i in range(mt_n):
                    nc.scalar.activation(sqscr[:], h_sb[:, i, :], AF.Square,
                                         bias=0.0, scale=1.0,
                                         accum_out=tsqm[:, i:i + 1])
                nc.vector.tensor_scalar(tsqm[:], tsqm[:], neg_t, None, OP.mult)

                # -t*sq over own columns (free layout), fp16 split
                hsq = ph1.tile([128, 2, m], F32)
                nc.vector.tensor_tensor(hsq[:], hT_sb[:], hT_sb[:], OP.mult)
                tsqr = ph1.tile([1, m], F32)
                with tc.tile_pool(name="psQ", bufs=2, space="PSUM") as psQ:
                    for c in range(ch_n1):
                        pq = psQ.tile([1, CH], F32, name="pq", bufs=2)
                        for dh in range(2):
                            nc.tensor.matmul(pq[:], ones128[:, 0:1],
                                             hsq[:, dh, c * CH:(c + 1) * CH],
                                             start=(dh == 0), stop=(dh == 1))
                        nc.scalar.copy(tsqr[:, c * CH:(c + 1) * CH], pq[:])
                nc.vector.tensor_scalar(tsqr[:], tsqr[:], neg_t, None, OP.mult)
                tsq_hi = ph1.tile([1, m], F16)
                tsq_hi32 = ph1.tile([1, m], F32)
                tsq_lo = ph1.tile([1, m], F16)
                nc.scalar.copy(tsq_hi[:], tsqr[:])
                nc.vector.tensor_copy(tsq_hi32[:], tsq_hi[:])
                nc.vector.tensor_tensor(tsq_lo[:], tsqr[:], tsq_hi32[:],
                                        OP.subtract)

                # fp16 split of hT (allgather payload = phase-2 rhs)
                hi16 = ph1.tile([128, 2, m], F16)
                lo16 = ph1.tile([128, 2, m], F16)
                hi32 = ph1.tile([128, 2, m], F32)
                nc.scalar.copy(hi16[:], hT_sb[:])
                nc.vector.tensor_copy(hi32[:], hi16[:])
                nc.vector.tensor_tensor(lo16[:], hT_sb[:], hi32[:], OP.subtract)

                # fp16 split of 2t*hT (phase-2 lhsT; robust for any t)
                th32 = ph1.tile([128, 2, m], F32)
                nc.vector.tensor_scalar(th32[:], hT_sb[:], two_t, None, OP.mult)
                nc.scalar.copy(thi[:], th32[:])
                nc.vector.tensor_copy(hi32[:], thi[:])
                nc.vector.tensor_tensor(tlo[:], th32[:], hi32[:], OP.subtract)

                # stage contribution, allgather
                nc.sync.dma_start(
                    agin[0, 0:D, :].rearrange("(dh p) m -> p dh m", p=128),
                    hi16[:])
                nc.sync.dma_start(
                    agin[1, 0:D, :].rearrange("(dh p) m -> p dh m", p=128),
                    lo16[:])
                nc.sync.dma_start(agin[0, D:D + 1, :], tsq_hi[:])
                nc.sync.dma_start(agin[1, D:D + 1, :], tsq_lo[:])
                with nc.named_scope("ag"):
                    nc.gpsimd.collective_compute(
                        "AllGather", OP.bypass, replica_groups=rg,
                        ins=[agin[:]], outs=[agout[:]],
                    )

            # ================= phase 2 =================
            with nc.named_scope("ph2"), \
                 tc.tile_pool(name="ph2", bufs=1) as ph2, \
                 tc.tile_pool(name="psZ", bufs=7, space="PSUM") as psZ, \
                 tc.tile_pool(name="psJ", bufs=1, space="PSUM") as psJ, \
                 tc.tile_pool(name="qp", bufs=2) as qp, \
                 tc.tile_pool(name="lp", bufs=2) as lp, \
                 tc.tile_pool(name="outp", bufs=2) as outp:
                rh = ph2.tile([128, 2, 2, n], F16)        # [p, split, dh, n]
                for s in range(2):
                    for dh in range(2):
                        nc.sync.dma_start(
                            rh[:, s, dh, :],
                            agout[:, s, dh * 128:(dh + 1) * 128, :]
                            .rearrange("c p m -> p c m"))
                augr = ph2.tile([2, n], F16)
                nc.sync.dma_start(augr[:],
                                  agout[:, :, D, :].rearrange("c s m -> s c m"))
                lq = ph2.tile([128, n], F32)
                junk = psJ.tile([128, CH], F32)
                # PE-warmth fillers: bridge the AllGather gap so the HAM
                # clock-gate stays at full rate into phase 2.
                for _ in range(144):
                    nc.tensor.matmul(junk[:], thi[:, 0, 0:128],
                                     thi[:, 0, 0:CH], start=True, stop=True)
                GC = 4                      # chunks per stationary-reuse group
                NQ = GC * CH // QW          # q tiles per group (2)
                for i in range(mt_n):
                    for g in range(ch_n2 // GC):
                        tqs = []
                        for qq in range(NQ):
                            qc = (g * GC) // (QW // CH) + qq
                            tq = qp.tile([128, QW], F32, name="tq", bufs=5)
                            nc.sync.dma_start(
                                tq[:],
                                qs[i * 128:(i + 1) * 128,
                                   qc * QW:(qc + 1) * QW])
                            tL = lp.tile([128, QW], F32, name="tL", bufs=1)
                            nc.scalar.activation(tL[:], tq[:], AF.Ln,
                                                 bias=eps[:], scale=1.0)
                            # g2 reuses the tq tile (tq dead after tL)
                            nc.scalar.activation(tq[:], tL[:], AF.Ln,
                                                 bias=0.0, scale=-1.0)
                            tqs.append(tq)
                        # double-bank psum tiles, one per q tile
                        pzs = [psZ.tile([128, QW], F32, name="pz", bufs=3)
                               for _ in range(NQ)]
                        # stationary-operand-outer order: one stationary load
                        # covers all GC chunks and both rh splits
                        for li, (lhs, srs) in enumerate(
                                ((thi, (0, 1)), (tlo, (0,)))):
                            for dh in range(2):
                                for sr in srs:
                                    for j in range(GC):
                                        c = g * GC + j
                                        pzv = pzs[(j * CH) // QW]
                                        off = (j * CH) % QW
                                        nc.tensor.matmul(
                                            pzv[:, off:off + CH],
                                            lhs[:, dh, i * 128:(i + 1) * 128],
                                            rh[:, sr, dh,
                                               c * CH:(c + 1) * CH],
                                            start=(li == 0 and dh == 0
                                                   and sr == 0),
                                            stop=False)
                        for j in range(GC):
                            c = g * GC + j
                            pzv = pzs[(j * CH) // QW]
                            off = (j * CH) % QW
                            nc.tensor.matmul(pzv[:, off:off + CH], ones2[:],
                                             augr[:, c * CH:(c + 1) * CH],
                                             start=False, stop=True)
                        for qq in range(NQ):
                            qc = (g * GC) // (QW // CH) + qq
                            # lq = min(z - t*sq_m, 0) - g2, one fused DVE pass
                            nc.vector._custom_dve(
                                MINSUB,
                                out=lq[:, qc * QW:(qc + 1) * QW],
                                in0=pzs[qq][:],
                                in1=tqs[qq][:],
                                s0=tsqm[:, i:i + 1])
                    tv8 = outp.tile([128, 8], F32, name="tv8", bufs=2)
                    ti8 = outp.tile([128, 8], U32, name="ti8", bufs=2)
                    nc.vector.max(tv8[:], lq[:])
                    nc.vector.max_index(ti8[:], tv8[:], lq[:])
                    nc.sync.dma_start(tv_out[i * 128:(i + 1) * 128, :], tv8[:])
                    nc.sync.dma_start(ti_out[i * 128:(i + 1) * 128, :], ti8[:])

    nc.finalize()
    return nc


def _host_prep(x, W, b, temperature, q, edge_index, n, m, n_cores):
    f32, f16 = np.float32, np.float16
    x = np.asarray(x, f32)
    W = np.asarray(W, f32)
    b = np.asarray(b, f32)
    q = np.asarray(q, f32)
    ei = np.asarray(edge_index).astype(np.int64)
    t = float(np.asarray(temperature))

    row = np.concatenate([ei[0], np.arange(n, dtype=np.int64)])  # dst
    col = np.concatenate([ei[1], np.arange(n, dtype=np.int64)])  # src
    deg = np.bincount(row, minlength=n).astype(f32)
    dinv = (1.0 / np.sqrt(np.maximum(deg, 1.0))).astype(f32)

    s = (dinv[:, None] * x).astype(f32)
    s_hi = s.astype(f16)
    s_lo = (s - s_hi.astype(f32)).astype(f16)

    at = np.zeros((n, n), dtype=np.uint8)        # AT[src, dst] multiplicity
    np.add.at(at, (col, row), 1)

    in_maps = []
    for c in range(n_cores):
        sl = slice(c * m, (c + 1) * m)
        p1c = np.empty((n, 512 + m), dtype=f16)
        p1c[:, 0:256] = s_hi
        p1c[:, 256:512] = s_lo
        p1c[:, 512:] = at[:, sl]
        in_maps.append({
            "p1": p1c,
            "wt": W,
            "bvec": b.reshape(D, 1),
            "dinvb": np.broadcast_to(dinv[sl][None, :], (128, m)).copy(),
            "qs": np.ascontiguousarray(q[sl, :]),
        })
    return in_maps, t


def kernel(x, W, b, temperature, q, edge_index):
    n, m, n_cores = N, M, NC
    in_maps, t = _host_prep(x, W, b, temperature, q, edge_index, n, m, n_cores)
    nc = build_program(n, m, two_t=2.0 * t, neg_t=-t, n_cores=n_cores)
    trace = bool(os.environ.get("DGM_TRACE"))
    res = run_bass_kernel_spmd(nc, in_maps, list(range(n_cores)), trace=trace)
    _LAST_RUN["res"] = res
    _LAST_RUN["nc"] = nc

    h = np.concatenate([res.results[c]["h_out"] for c in range(n_cores)], axis=0)
    tv8 = np.concatenate([res.results[c]["tv_out"] for c in range(n_cores)], axis=0)
    ti8 = np.concatenate([res.results[c]["ti_out"] for c in range(n_cores)], axis=0)
    top_vals = np.ascontiguousarray(tv8[:, :KNN]).astype(np.float32)
    top_idx = np.ascontiguousarray(ti8[:, :KNN]).astype(np.int32)
    edges = np.stack([top_idx.reshape(-1),
                      np.repeat(np.arange(n, dtype=np.int32), KNN)])
    return h, edges, top_vals
